# revision 4
# baseline (speedup 1.0000x reference)
"""Trainium2 Bass kernel for the DGL-style ChebConv GNN classifier.

Strategy (8 NeuronCores, SPMD):
  - Nodes sharded contiguously across cores (12.5K rows each); edges owned by
    the core that owns their dst.
  - Per ChebConv layer, the Laplacian application ahat() is computed twice
    (Chebyshev K=3) as: gather src rows from a replicated node table
    (dma_gather, int16 indices over 32K-row chunks), then segment-sum via
    one-hot matmuls accumulating in PSUM, evacuated with per-partition norm
    scalings.  Work happens in "Y = X * norm" space so every scaling is a
    per-dst-row (per-partition) tensor_scalar.
  - Node tables are re-replicated between passes with AllGather collectives.
  - The dense (concat @ W) matmuls consume PE-transposed feature-major
    blocks; relu(+scale) epilogue on the scalar engine writes the next
    layer's table shard.
  - Readout: per-core one-hot (graph id) matmul partial sums + AllReduce,
    then the small MLP classifier on-chip.

kernel(**inputs) takes FULL unsharded inputs and returns the FULL [G, 10]
output; all sharding happens inside.
"""

import math
import os

import numpy as np

import concourse.bass as bass
import concourse.bacc as bacc
import concourse.mybir as mybir
import concourse.tile as tile
from concourse.bass_utils import run_bass_kernel_spmd

NCORES = 8
P = 128
CHUNK = 32768          # int16 index range for dma_gather
BB = 4                 # dst blocks per batch (bounds live scatter-psum banks)
MAX_CALL = 1024        # max slots per dma_gather call (SWDGE carveout = 1024 descs)
F32 = mybir.dt.float32
I16 = mybir.dt.int16


def _wrap16(local_idx):
    """[L] -> [128, L/16]: element i at [i%16, i//16], replicated to 128
    partitions (8 Q7 cores each read a 16-partition group)."""
    L = local_idx.shape[0]
    w = local_idx.reshape(L // 16, 16).T.copy()
    return np.tile(w, (8, 1))


def _preprocess(src, dst, graph_ids, N, G):
    """Build the shared (SPMD-equal) program structure + per-core data."""
    E = src.shape[0]
    NLOC = (N + NCORES - 1) // NCORES
    NB = (NLOC + P - 1) // P            # dst blocks per core
    NBATCH = (NB + BB - 1) // BB
    NCH = (N + CHUNK - 1) // CHUNK

    deg = np.bincount(dst, minlength=N).astype(np.float32)
    norm = np.clip(deg, 1.0, None) ** -0.5          # [N]
    norm2 = norm * norm
    inv_norm = 1.0 / norm

    # ---- per-core edge streams ----------------------------------------
    core_of = dst // NLOC
    per_core = []
    counts = np.zeros((NCORES, NBATCH, NCH), dtype=np.int64)
    for c in range(NCORES):
        m = core_of == c
        s = src[m]
        dl = dst[m] - c * NLOC
        blk = dl // P
        bat = blk // BB
        ch = s // CHUNK
        order = np.lexsort((dl, ch, bat))
        s, dl, bat, ch = s[order], dl[order], bat[order], ch[order]
        key = bat * NCH + ch
        cnt = np.bincount(key, minlength=NBATCH * NCH).reshape(NBATCH, NCH)
        counts[c] = cnt
        per_core.append((s, dl, key))

    # equalized run lengths (128-aligned), shared across cores
    runlen = (
        ((counts.max(axis=0) + P - 1) // P) * P
    )  # [NBATCH, NCH]
    run_off = np.zeros((NBATCH, NCH), dtype=np.int64)
    tot = 0
    for t in range(NBATCH):
        for ch in range(NCH):
            run_off[t, ch] = tot
            tot += runlen[t, ch]
    TOT = int(tot)
    NSUB = TOT // P

    # ---- slot arrays per core -----------------------------------------
    slot_src = np.empty((NCORES, TOT), dtype=np.int64)
    slot_dstl = np.empty((NCORES, TOT), dtype=np.int64)
    for c in range(NCORES):
        s, dl, key = per_core[c]
        ssrc = np.empty(TOT, dtype=np.int64)
        sdst = np.full(TOT, -1, dtype=np.int64)
        # fill pads with the chunk's base row (valid gather, zero one-hot)
        for t in range(NBATCH):
            for ch in range(NCH):
                o, L = run_off[t, ch], runlen[t, ch]
                ssrc[o : o + L] = ch * CHUNK if ch * CHUNK < N else 0
        pos = np.empty(len(key), dtype=np.int64)
        # edges are sorted by key; place each run's edges at its offset
        kcnt = counts[c].reshape(-1)
        koff = run_off.reshape(-1)
        start = 0
        for k in range(NBATCH * NCH):
            n = kcnt[k]
            pos[start : start + n] = koff[k] + np.arange(n)
            start += n
        ssrc[pos] = s
        sdst[pos] = dl
        slot_src[c] = ssrc
        slot_dstl[c] = sdst

    # ---- gather calls (shared structure) ------------------------------
    # each call: one (batch, chunk) run split into <=MAX_CALL slot segments
    calls = []  # (chunk, slot_off, length)
    for t in range(NBATCH):
        for ch in range(NCH):
            o, L = int(run_off[t, ch]), int(runlen[t, ch])
            while L > 0:
                seg = min(L, MAX_CALL)
                calls.append((ch, o, seg))
                o += seg
                L -= seg

    # idx16_all: [128, TOT/16] int16 per core, per-call wrap
    idx16 = np.zeros((NCORES, P, TOT // 16), dtype=np.int16)
    for c in range(NCORES):
        for ch, o, L in calls:
            local = (slot_src[c, o : o + L] - ch * CHUNK).astype(np.int16)
            idx16[c][:, o // 16 : (o + L) // 16] = _wrap16(local)

    # map subtile -> (call index, column within call)
    sub_call = np.empty(NSUB, dtype=np.int64)
    sub_col = np.empty(NSUB, dtype=np.int64)
    for k, (ch, o, L) in enumerate(calls):
        for j in range(L // P):
            sub_call[o // P + j] = k
            sub_col[o // P + j] = j

    # ---- (subtile, block) pairs: union across cores -------------------
    blk_all = slot_dstl // P  # [NCORES, TOT], -1 for pads
    pairs = []  # (subtile, block)
    for sidx in range(NSUB):
        sl = blk_all[:, sidx * P : (sidx + 1) * P]
        present = np.unique(sl[sl >= 0])
        for b in present:
            pairs.append((sidx, int(b)))
    NPAIRS = len(pairs)

    # per-block pair index ranges (first/last occurrence in pair order)
    first_pair = {}
    last_pair = {}
    for j, (sidx, b) in enumerate(pairs):
        if b not in first_pair:
            first_pair[b] = j
        last_pair[b] = j

    # dstsel: [128, NPAIRS] fp32 per core
    dstsel = np.full((NCORES, P, NPAIRS), -1.0, dtype=np.float32)
    for j, (sidx, b) in enumerate(pairs):
        sl = slot_dstl[:, sidx * P : (sidx + 1) * P]  # [NCORES, 128]
        m = (sl // P) == b
        col = np.where(m, (sl - b * P).astype(np.float32), -1.0)
        dstsel[:, :, j] = col

    # ---- per-block norm columns & graph sel ---------------------------
    # normc[c]: [128, 4*NB]: kinds (0: -norm2, 1: -2*norm2, 2: inv_norm, 3: norm)
    normc = np.zeros((NCORES, P, 4 * NB), dtype=np.float32)
    gsel = np.full((NCORES, P, NB), -1.0, dtype=np.float32)
    for c in range(NCORES):
        lo = c * NLOC
        hi = min(lo + NLOC, N)
        n = hi - lo
        pad = NB * P - n
        nn = np.pad(norm[lo:hi], (0, pad)).reshape(NB, P).T
        n2 = np.pad(norm2[lo:hi], (0, pad)).reshape(NB, P).T
        iv = np.pad(inv_norm[lo:hi], (0, pad)).reshape(NB, P).T
        normc[c][:, 0 * NB : 1 * NB] = -n2
        normc[c][:, 1 * NB : 2 * NB] = -2.0 * n2
        normc[c][:, 2 * NB : 3 * NB] = iv
        normc[c][:, 3 * NB : 4 * NB] = nn
        gs = np.pad(graph_ids[lo:hi].astype(np.float32), (0, pad), constant_values=-1.0)
        gsel[c] = gs.reshape(NB, P).T

    block_rows = [min(P, NLOC - b * P) for b in range(NB)]

    return dict(
        N=N, E=E, G=G, NLOC=NLOC, NB=NB, NBATCH=NBATCH, NCH=NCH,
        TOT=TOT, NSUB=NSUB, calls=calls, pairs=pairs,
        first_pair=first_pair, last_pair=last_pair,
        sub_call=sub_call, sub_col=sub_col,
        idx16=idx16, dstsel=dstsel, normc=normc, gsel=gsel,
        block_rows=block_rows, norm=norm,
    )


DIN = [128, 128, 128, 256]
DOUT = [128, 128, 256, 512]


def _build(S):
    """Build the SPMD Bass program (shared across cores)."""
    KSTAGE = int(os.environ.get("KSTAGE", "99"))
    KSINGLE = os.environ.get("KSINGLE", "0") == "1"
    NLOC, NB, NCH, TOT = S["NLOC"], S["NB"], S["NCH"], S["TOT"]
    NPAIRS = len(S["pairs"])
    NTAB = NCORES * NLOC  # table rows (>= N)

    nc = bacc.Bacc(trn_type="TRN2", num_devices=1 if KSINGLE else NCORES,
                   dynamic_dma_scratch_size=49152, num_swdge_queues=4)

    sig_in = nc.dram_tensor("sig", [NLOC, 128], F32, kind="ExternalInput")
    idx_in = nc.dram_tensor("idx16", [P, TOT // 16], I16, kind="ExternalInput")
    dsel_in = nc.dram_tensor("dstsel", [P, NPAIRS], F32, kind="ExternalInput")
    normc_in = nc.dram_tensor("normc", [P, 4 * NB], F32, kind="ExternalInput")
    gsel_in = nc.dram_tensor("gsel", [P, NB], F32, kind="ExternalInput")
    iota_in = nc.dram_tensor("iota", [P, P], F32, kind="ExternalInput")
    ident_in = nc.dram_tensor("ident", [P, P], F32, kind="ExternalInput")
    w_in = [
        nc.dram_tensor(f"W{l}", [3 * DIN[l], DOUT[l]], F32, kind="ExternalInput")
        for l in range(4)
    ]
    bt_in = [
        nc.dram_tensor(f"Bt{l}", [P, DOUT[l]], F32, kind="ExternalInput")
        for l in range(4)
    ]
    wm1_in = nc.dram_tensor("Wm1", [512, 512], F32, kind="ExternalInput")
    bm1_in = nc.dram_tensor("Bm1", [P, 512], F32, kind="ExternalInput")
    wm2_in = nc.dram_tensor("Wm2", [512, 16], F32, kind="ExternalInput")
    bm2_in = nc.dram_tensor("Bm2", [P, 16], F32, kind="ExternalInput")
    out = nc.dram_tensor("out", [P, 16], F32, kind="ExternalOutput")

    with tile.TileContext(nc) as tc:
        with (
            tc.tile_pool(name="dram", bufs=1, space="DRAM") as dram,
            tc.tile_pool(name="res", bufs=1) as res,
            tc.tile_pool(name="sb", bufs=3) as sb,
            tc.tile_pool(name="scp", bufs=1, space="PSUM") as scp,
            tc.tile_pool(name="pp", bufs=2, space="PSUM") as pp,
            tc.tile_pool(name="tpp", bufs=1, space="PSUM") as tpp,
            tc.tile_pool(name="rdp", bufs=1, space="PSUM") as rdp,
        ):
            # ------- resident metadata -------
            idx_sb = res.tile([P, TOT // 16], I16)
            dsel_sb = res.tile([P, NPAIRS], F32)
            normc_sb = res.tile([P, 4 * NB], F32)
            gsel_sb = res.tile([P, NB], F32)
            iota_sb = res.tile([P, P], F32)
            ident_sb = res.tile([P, P], F32)
            nc.sync.dma_start(out=idx_sb[:], in_=idx_in[:, :])
            nc.sync.dma_start(out=dsel_sb[:], in_=dsel_in[:, :])
            nc.sync.dma_start(out=normc_sb[:], in_=normc_in[:, :])
            nc.sync.dma_start(out=gsel_sb[:], in_=gsel_in[:, :])
            nc.sync.dma_start(out=iota_sb[:], in_=iota_in[:, :])
            nc.sync.dma_start(out=ident_sb[:], in_=ident_in[:, :])
            w_sb = []
            for l in range(4):
                nchk = 3 * DIN[l] // P
                t = res.tile([P, nchk * DOUT[l]], F32, tag=f"W{l}")
                for j in range(nchk):
                    nc.sync.dma_start(
                        out=t[:, j * DOUT[l] : (j + 1) * DOUT[l]],
                        in_=w_in[l][j * P : (j + 1) * P, :],
                    )
                w_sb.append(t)
            bt_sb = []
            for l in range(4):
                t = res.tile([P, DOUT[l]], F32, tag=f"Bt{l}")
                nc.sync.dma_start(out=t[:], in_=bt_in[l][:, :])
                bt_sb.append(t)
            wm1_sb = res.tile([P, 4 * 512], F32)
            for j in range(4):
                nc.sync.dma_start(
                    out=wm1_sb[:, j * 512 : (j + 1) * 512],
                    in_=wm1_in[j * P : (j + 1) * P, :],
                )
            bm1_sb = res.tile([P, 512], F32)
            nc.sync.dma_start(out=bm1_sb[:], in_=bm1_in[:, :])
            wm2_sb = res.tile([P, 4 * 16], F32)
            for j in range(4):
                nc.sync.dma_start(
                    out=wm2_sb[:, j * 16 : (j + 1) * 16],
                    in_=wm2_in[j * P : (j + 1) * P, :],
                )
            bm2_sb = res.tile([P, 16], F32)
            nc.sync.dma_start(out=bm2_sb[:], in_=bm2_in[:, :])

            # ------- DRAM tables -------
            y0s = [dram.tile([NLOC, DIN[l]], F32, tag=f"y0s{l}", name=f"y0s{l}") for l in range(4)]
            y1s = [dram.tile([NLOC, DIN[l]], F32, tag=f"y1s{l}", name=f"y1s{l}") for l in range(4)]
            y0f = [dram.tile([NTAB, DIN[l]], F32, tag=f"y0f{l}", name=f"y0f{l}", addr_space="Shared") for l in range(4)]
            y1f = [dram.tile([NTAB, DIN[l]], F32, tag=f"y1f{l}", name=f"y1f{l}", addr_space="Shared") for l in range(4)]

            RG = [list(range(NCORES))]

            def ag(shard, full):
                if KSINGLE:
                    nc.sync.dma_start(out=full[0 : shard.shape[0], :], in_=shard[:])
                    return
                nc.gpsimd.collective_compute(
                    "AllGather", mybir.AluOpType.bypass,
                    replica_groups=RG, ins=[shard[:]], outs=[full[:]],
                )

            def ncol(kind, b):
                return normc_sb[:, kind * NB + b : kind * NB + b + 1]

            # ------- P0: Y0_0 = signal * norm -------
            for b in range(NB):
                rows = S["block_rows"][b]
                sg = sb.tile([P, 128], F32, tag="sg")
                if rows < P:
                    nc.any.memset(sg[:], 0.0)
                nc.sync.dma_start(out=sg[:rows], in_=sig_in[b * P : b * P + rows, :])
                o = sb.tile([P, 128], F32, tag="p0o")
                nc.any.tensor_scalar(
                    out=o[:], in0=sg[:], scalar1=ncol(3, b), scalar2=None,
                    op0=mybir.AluOpType.mult,
                )
                nc.sync.dma_start(out=y0s[0][b * P : b * P + rows, :], in_=o[:rows])
            ag(y0s[0], y0f[0])

            def dump_and_stop(tab):
                d = sb.tile([P, 16], F32, tag="dmp", name="dmp")
                nc.sync.dma_start(out=d[:], in_=tab[0:P, 0:16])
                nc.sync.dma_start(out=out[:, :], in_=d[:])

            stopped = KSTAGE == 0
            if stopped:
                dump_and_stop(y0f[0])

            # ------- scatter unit -------
            def scatter_unit(table, D, evac_fn):
                g_tiles = {}
                psums = {}
                emitted = -1
                for j, (sidx, b) in enumerate(S["pairs"]):
                    k = int(S["sub_call"][sidx])
                    if k > emitted:
                        for kk in range(emitted + 1, k + 1):
                            ch, o, L = S["calls"][kk]
                            rows_ch = min(CHUNK, NTAB - ch * CHUNK)
                            g = sb.tile([P, (MAX_CALL // P) * 256], F32, tag="g", bufs=3, name="g")
                            nc.gpsimd.dma_gather(
                                out_ap=g[:, : (L // P) * D].rearrange(
                                    "p (k d) -> p k d", d=D
                                ),
                                in_ap=table[ch * CHUNK : ch * CHUNK + rows_ch, :],
                                idxs_ap=idx_sb[:, o // 16 : (o + L) // 16],
                                num_idxs=L,
                                num_idxs_reg=L,
                                elem_size=D,
                                queue_num=kk % 4,
                            )
                            g_tiles[kk] = g
                        emitted = k
                    col = int(S["sub_col"][sidx])
                    s_t = sb.tile([P, P], F32, tag="s", bufs=8, name="s_t")
                    if j % 3 == 0:
                        # ACT path: onehot = Relu(1 - Abs(dsel - iota))
                        a_t = sb.tile([P, P], F32, tag="oha", bufs=4, name="a_t")
                        nc.scalar.activation(
                            a_t[:], iota_sb[:],
                            mybir.ActivationFunctionType.Abs,
                            bias=dsel_sb[:, j : j + 1], scale=-1.0,
                        )
                        nc.scalar.activation(
                            s_t[:], a_t[:],
                            mybir.ActivationFunctionType.Relu,
                            bias=1.0, scale=-1.0,
                        )
                    else:
                        nc.vector.tensor_scalar(
                            out=s_t[:], in0=iota_sb[:],
                            scalar1=dsel_sb[:, j : j + 1], scalar2=None,
                            op0=mybir.AluOpType.is_equal,
                        )
                    if b not in psums:
                        psums[b] = scp.tile([P, D], F32, tag=f"sc{b % BB}", name=f"scps{b % BB}")
                    nc.tensor.matmul(
                        out=psums[b][:],
                        lhsT=s_t[:],
                        rhs=g_tiles[k][:, col * D : (col + 1) * D],
                        start=(j == S["first_pair"][b]),
                        stop=(j == S["last_pair"][b]),
                    )
                    if j == S["last_pair"][b]:
                        evac_fn(b, psums.pop(b))

            # ------- layers -------
            psum_r = rdp.tile([P, 512], F32, tag="rd")
            for l in range(4):
                if stopped:
                    break
                D = DIN[l]
                nkc = D // P

                def evac_a(b, ps, l=l):
                    rows = S["block_rows"][b]
                    ev = sb.tile([P, D], F32, tag="ev")
                    nc.any.tensor_scalar(
                        out=ev[:], in0=ps[:], scalar1=ncol(0, b), scalar2=None,
                        op0=mybir.AluOpType.mult,
                    )
                    nc.sync.dma_start(
                        out=y1s[l][b * P : b * P + rows, :], in_=ev[:rows]
                    )

                scatter_unit(y0f[l], D, evac_a)
                ag(y1s[l], y1f[l])
                if KSTAGE == 10 + l:
                    dump_and_stop(y1f[l])
                    stopped = True
                    break

                def evac_b(b, ps, l=l, nkc=nkc, D=D):
                    rows = S["block_rows"][b]
                    tb = sb.tile([P, D], F32, tag="tb")
                    nc.any.tensor_scalar(
                        out=tb[:], in0=ps[:], scalar1=ncol(1, b), scalar2=None,
                        op0=mybir.AluOpType.mult,
                    )
                    y0b = sb.tile([P, D], F32, tag="y0b")
                    y1b = sb.tile([P, D], F32, tag="y1b")
                    if rows < P:
                        nc.any.memset(y0b[:], 0.0)
                        nc.any.memset(y1b[:], 0.0)
                    nc.sync.dma_start(
                        out=y0b[:rows], in_=y0s[l][b * P : b * P + rows, :]
                    )
                    nc.sync.dma_start(
                        out=y1b[:rows], in_=y1s[l][b * P : b * P + rows, :]
                    )
                    y2b = sb.tile([P, D], F32, tag="y2b")
                    nc.any.tensor_tensor(
                        out=y2b[:], in0=tb[:], in1=y0b[:],
                        op=mybir.AluOpType.subtract,
                    )
                    # transposes -> feature-major lhsT chunks
                    yts = []
                    for term, ysrc in enumerate((y0b, y1b, y2b)):
                        for kc in range(nkc):
                            tp = tpp.tile([P, P], F32, tag="tp")
                            nc.tensor.transpose(
                                out=tp[:],
                                in_=ysrc[:, kc * P : (kc + 1) * P],
                                identity=ident_sb[:],
                            )
                            yt = sb.tile([P, P], F32, tag="yt", bufs=8, name="yt")
                            nc.any.tensor_copy(out=yt[:], in_=tp[:])
                            yts.append(yt)
                    ph = pp.tile([P, DOUT[l]], F32, tag="dh")
                    nchk = 3 * nkc
                    for j2 in range(nchk):
                        nc.tensor.matmul(
                            out=ph[:],
                            lhsT=yts[j2][:],
                            rhs=w_sb[l][:, j2 * DOUT[l] : (j2 + 1) * DOUT[l]],
                            start=(j2 == 0),
                            stop=(j2 == nchk - 1),
                        )
                    t1 = sb.tile([P, DOUT[l]], F32, tag="t1")
                    nc.any.tensor_scalar(
                        out=t1[:], in0=ph[:], scalar1=ncol(2, b), scalar2=None,
                        op0=mybir.AluOpType.mult,
                    )
                    t2 = sb.tile([P, DOUT[l]], F32, tag="t2")
                    nc.any.tensor_tensor(
                        out=t2[:], in0=t1[:], in1=bt_sb[l][:],
                        op=mybir.AluOpType.add,
                    )
                    if l < 3:
                        o = sb.tile([P, DOUT[l]], F32, tag="lo")
                        nc.scalar.activation(
                            o[:], t2[:], mybir.ActivationFunctionType.Relu,
                            scale=ncol(3, b),
                        )
                        nc.sync.dma_start(
                            out=y0s[l + 1][b * P : b * P + rows, :], in_=o[:rows]
                        )
                    else:
                        h = sb.tile([P, 512], F32, tag="h")
                        nc.scalar.activation(
                            h[:], t2[:], mybir.ActivationFunctionType.Relu,
                        )
                        gs = sb.tile([P, P], F32, tag="gs")
                        nc.any.tensor_scalar(
                            out=gs[:], in0=iota_sb[:],
                            scalar1=gsel_sb[:, b : b + 1], scalar2=None,
                            op0=mybir.AluOpType.is_equal,
                        )
                        nc.tensor.matmul(
                            out=psum_r[:],
                            lhsT=gs[:],
                            rhs=h[:],
                            start=(b == 0),
                            stop=(b == NB - 1),
                        )

                scatter_unit(y1f[l], D, evac_b)
                if l < 3:
                    ag(y0s[l + 1], y0f[l + 1])
                if KSTAGE == 20 + l:
                    if l < 3:
                        dump_and_stop(y0f[l + 1])
                    stopped = True
                    break

            # ------- readout + MLP -------
            if stopped:
                hgp = None
            else:
                hgp = sb.tile([P, 512], F32, tag="hgp")
            if not stopped:
                nc.any.tensor_copy(out=hgp[:], in_=psum_r[:])
                part_d = dram.tile([P, 512], F32, tag="part")
                tot_d = dram.tile([P, 512], F32, tag="tot")
                nc.sync.dma_start(out=part_d[:], in_=hgp[:])
                if KSINGLE:
                    nc.sync.dma_start(out=tot_d[:], in_=part_d[:])
                else:
                    nc.gpsimd.collective_compute(
                        "AllReduce", mybir.AluOpType.add,
                        replica_groups=RG, ins=[part_d[:]], outs=[tot_d[:]],
                    )
                hg = sb.tile([P, 512], F32, tag="hg")
                nc.sync.dma_start(out=hg[:], in_=tot_d[:])

            def mm_block(x_sb, wtile, dout, bias_tile, relu):
                # out = act(x @ W + b): x [128, 512] -> 4 transposed chunks
                xts = []
                for kc in range(4):
                    tp = tpp.tile([P, P], F32, tag="tp")
                    nc.tensor.transpose(
                        out=tp[:], in_=x_sb[:, kc * P : (kc + 1) * P],
                        identity=ident_sb[:],
                    )
                    xt = sb.tile([P, P], F32, tag="yt", bufs=8, name="xt")
                    nc.any.tensor_copy(out=xt[:], in_=tp[:])
                    xts.append(xt)
                ph = pp.tile([P, dout], F32, tag="dh")
                for kc in range(4):
                    nc.tensor.matmul(
                        out=ph[:], lhsT=xts[kc][:],
                        rhs=wtile[:, kc * dout : (kc + 1) * dout],
                        start=(kc == 0), stop=(kc == 3),
                    )
                o = sb.tile([P, dout], F32, tag=f"mo{dout}")
                nc.any.tensor_tensor(
                    out=o[:], in0=ph[:], in1=bias_tile[:], op=mybir.AluOpType.add
                )
                if relu:
                    r = sb.tile([P, dout], F32, tag=f"mr{dout}")
                    nc.scalar.activation(
                        r[:], o[:], mybir.ActivationFunctionType.Relu
                    )
                    return r
                return o

            if not stopped:
                m1 = mm_block(hg, wm1_sb, 512, bm1_sb, relu=True)
                m2 = mm_block(m1, wm2_sb, 16, bm2_sb, relu=False)
                nc.sync.dma_start(out=out[:, :], in_=m2[:])

    nc.finalize()
    return nc


_CACHE = {}
G_OVERRIDE = None      # test hook (reference uses G=128)
TRACE = False          # test hook: request NTFF profiling
LAST_RESULTS = None    # test hook: BassKernelResults of the last run


def kernel(signal, W0, b0, W1, b1, W2, b2, W3, b3, Wm1, bm1, Wm2, bm2,
           src, dst, graph_ids):
    global LAST_RESULTS
    signal = np.ascontiguousarray(np.asarray(signal, dtype=np.float32))
    src = np.asarray(src).astype(np.int64)
    dst = np.asarray(dst).astype(np.int64)
    graph_ids = np.asarray(graph_ids).astype(np.int64)
    N = signal.shape[0]
    G = G_OVERRIDE or 128

    key = (N, src.shape[0], G, hash(src.tobytes()) ^ hash(dst.tobytes())
           ^ hash(graph_ids.tobytes()))
    if key in _CACHE:
        S, nc = _CACHE[key]
    else:
        S = _preprocess(src, dst, graph_ids, N, G)
        nc = _build(S)
        _CACHE.clear()
        _CACHE[key] = (S, nc)

    in_maps = _make_inmaps(
        S, signal, W0, b0, W1, b1, W2, b2, W3, b3, Wm1, bm1, Wm2, bm2
    )

    res = run_bass_kernel_spmd(
        nc, in_maps, core_ids=list(range(NCORES)), trace=TRACE
    )
    LAST_RESULTS = res
    return np.asarray(res.results[0]["out"][:G, :10])


def _make_inmaps(S, signal, W0, b0, W1, b1, W2, b2, W3, b3, Wm1, bm1, Wm2, bm2):
    N = signal.shape[0]
    NLOC = S["NLOC"]
    iota_np = np.broadcast_to(
        np.arange(P, dtype=np.float32)[None, :], (P, P)
    ).copy()
    ident_np = np.eye(P, dtype=np.float32)
    ws = [np.asarray(w, dtype=np.float32) for w in (W0, W1, W2, W3)]
    bts = [
        np.broadcast_to(np.asarray(b, dtype=np.float32)[None, :], (P, len(b))).copy()
        for b in (b0, b1, b2, b3)
    ]
    wm2_p = np.zeros((512, 16), np.float32)
    wm2_p[:, :10] = np.asarray(Wm2, dtype=np.float32)
    bm2_p = np.zeros((P, 16), np.float32)
    bm2_p[:, :10] = np.asarray(bm2, dtype=np.float32)[None, :]
    bm1_t = np.broadcast_to(
        np.asarray(bm1, dtype=np.float32)[None, :], (P, 512)
    ).copy()

    in_maps = []
    for c in range(NCORES):
        lo = c * NLOC
        hi = min(N, lo + NLOC)
        shard = np.zeros((NLOC, 128), np.float32)
        shard[: hi - lo] = signal[lo:hi]
        m = {
            "sig": shard,
            "idx16": S["idx16"][c],
            "dstsel": S["dstsel"][c],
            "normc": S["normc"][c],
            "gsel": S["gsel"][c],
            "iota": iota_np,
            "ident": ident_np,
            "Wm1": np.asarray(Wm1, dtype=np.float32),
            "Bm1": bm1_t,
            "Wm2": wm2_p,
            "Bm2": bm2_p,
        }
        for l in range(4):
            m[f"W{l}"] = ws[l]
            m[f"Bt{l}"] = bts[l]
        in_maps.append(m)
    return in_maps



# revision 20
# speedup vs baseline: 1.4422x; 1.4422x over previous
"""Trainium2 Bass kernel for the DGL-style ChebConv GNN classifier.

Strategy (8 NeuronCores, SPMD):
  - Nodes sharded contiguously across cores (12.5K rows each); edges owned by
    the core that owns their dst.
  - Per ChebConv layer, the Laplacian application ahat() is computed twice
    (Chebyshev K=3) as: gather src rows from a replicated node table
    (dma_gather, int16 indices over 32K-row chunks), then segment-sum via
    one-hot matmuls accumulating in PSUM, evacuated with per-partition norm
    scalings.  Work happens in "Y = X * norm" space so every scaling is a
    per-dst-row (per-partition) tensor_scalar.
  - Node tables are re-replicated between passes with AllGather collectives.
  - The dense (concat @ W) matmuls consume PE-transposed feature-major
    blocks; relu(+scale) epilogue on the scalar engine writes the next
    layer's table shard.
  - Readout: per-core one-hot (graph id) matmul partial sums + AllReduce,
    then the small MLP classifier on-chip.

kernel(**inputs) takes FULL unsharded inputs and returns the FULL [G, 10]
output; all sharding happens inside.
"""

import math
import os

import numpy as np

import concourse.bass as bass
import concourse.bacc as bacc
import concourse.mybir as mybir
import concourse.tile as tile
from concourse.bass_utils import run_bass_kernel_spmd

NCORES = 8
P = 128
CHUNK = 32768          # int16 index range for dma_gather
BB = 4                 # dst blocks per batch (bounds live scatter-psum banks)
MAX_CALL = 1024        # max slots per dma_gather call (SWDGE carveout = 1024 descs)
F32 = mybir.dt.float32
BF = mybir.dt.bfloat16
I16 = mybir.dt.int16


def _wrap16(local_idx):
    """[L] -> [128, L/16]: element i at [i%16, i//16], replicated to 128
    partitions (8 Q7 cores each read a 16-partition group)."""
    L = local_idx.shape[0]
    w = local_idx.reshape(L // 16, 16).T.copy()
    return np.tile(w, (8, 1))


def _preprocess(src, dst, graph_ids, N, G):
    """Build the shared (SPMD-equal) program structure + per-core data."""
    E = src.shape[0]
    NLOC = (N + NCORES - 1) // NCORES
    NB = (NLOC + P - 1) // P            # dst blocks per core
    NBATCH = (NB + BB - 1) // BB
    NCH = (N + CHUNK - 1) // CHUNK

    deg = np.bincount(dst, minlength=N).astype(np.float32)
    norm = np.clip(deg, 1.0, None) ** -0.5          # [N]
    norm2 = norm * norm
    inv_norm = 1.0 / norm

    # ---- per-core edge streams ----------------------------------------
    core_of = dst // NLOC
    per_core = []
    counts = np.zeros((NCORES, NBATCH, NCH), dtype=np.int64)
    for c in range(NCORES):
        m = core_of == c
        s = src[m]
        dl = dst[m] - c * NLOC
        blk = dl // P
        bat = blk // BB
        ch = s // CHUNK
        order = np.lexsort((dl, ch, bat))
        s, dl, bat, ch = s[order], dl[order], bat[order], ch[order]
        key = bat * NCH + ch
        cnt = np.bincount(key, minlength=NBATCH * NCH).reshape(NBATCH, NCH)
        counts[c] = cnt
        per_core.append((s, dl, key))

    # equalized run lengths (128-aligned), shared across cores
    runlen = (
        ((counts.max(axis=0) + P - 1) // P) * P
    )  # [NBATCH, NCH]
    run_off = np.zeros((NBATCH, NCH), dtype=np.int64)
    tot = 0
    for t in range(NBATCH):
        for ch in range(NCH):
            run_off[t, ch] = tot
            tot += runlen[t, ch]
    TOT = int(tot)
    NSUB = TOT // P

    # ---- slot arrays per core -----------------------------------------
    slot_src = np.empty((NCORES, TOT), dtype=np.int64)
    slot_dstl = np.empty((NCORES, TOT), dtype=np.int64)
    for c in range(NCORES):
        s, dl, key = per_core[c]
        ssrc = np.empty(TOT, dtype=np.int64)
        sdst = np.full(TOT, -1, dtype=np.int64)
        # fill pads with the chunk's base row (valid gather, zero one-hot)
        for t in range(NBATCH):
            for ch in range(NCH):
                o, L = run_off[t, ch], runlen[t, ch]
                ssrc[o : o + L] = ch * CHUNK if ch * CHUNK < N else 0
        pos = np.empty(len(key), dtype=np.int64)
        # edges are sorted by key; place each run's edges at its offset
        kcnt = counts[c].reshape(-1)
        koff = run_off.reshape(-1)
        start = 0
        for k in range(NBATCH * NCH):
            n = kcnt[k]
            pos[start : start + n] = koff[k] + np.arange(n)
            start += n
        ssrc[pos] = s
        sdst[pos] = dl
        slot_src[c] = ssrc
        slot_dstl[c] = sdst

    # ---- gather calls (shared structure) ------------------------------
    # each call: one (batch, chunk) run split into <=MAX_CALL slot segments
    calls = []  # (chunk, slot_off, length)
    for t in range(NBATCH):
        for ch in range(NCH):
            o, L = int(run_off[t, ch]), int(runlen[t, ch])
            while L > 0:
                seg = min(L, MAX_CALL)
                calls.append((ch, o, seg))
                o += seg
                L -= seg

    # idx16_all: [128, TOT/16] int16 per core, per-call wrap
    idx16 = np.zeros((NCORES, P, TOT // 16), dtype=np.int16)
    for c in range(NCORES):
        for ch, o, L in calls:
            local = (slot_src[c, o : o + L] - ch * CHUNK).astype(np.int16)
            idx16[c][:, o // 16 : (o + L) // 16] = _wrap16(local)

    # map subtile -> (call index, column within call)
    sub_call = np.empty(NSUB, dtype=np.int64)
    sub_col = np.empty(NSUB, dtype=np.int64)
    for k, (ch, o, L) in enumerate(calls):
        for j in range(L // P):
            sub_call[o // P + j] = k
            sub_col[o // P + j] = j

    # ---- (subtile, block) pairs: union across cores -------------------
    blk_all = slot_dstl // P  # [NCORES, TOT], -1 for pads
    pairs = []  # (subtile, block)
    for sidx in range(NSUB):
        sl = blk_all[:, sidx * P : (sidx + 1) * P]
        present = np.unique(sl[sl >= 0])
        for b in present:
            pairs.append((sidx, int(b)))
    NPAIRS = len(pairs)

    # per-block pair index ranges (first/last occurrence in pair order)
    first_pair = {}
    last_pair = {}
    for j, (sidx, b) in enumerate(pairs):
        if b not in first_pair:
            first_pair[b] = j
        last_pair[b] = j

    # dstsel: [128, NPAIRS] fp32 per core
    dstsel = np.full((NCORES, P, NPAIRS), -1.0, dtype=np.float32)
    for j, (sidx, b) in enumerate(pairs):
        sl = slot_dstl[:, sidx * P : (sidx + 1) * P]  # [NCORES, 128]
        m = (sl // P) == b
        col = np.where(m, (sl - b * P).astype(np.float32), -1.0)
        dstsel[:, :, j] = col

    # ---- per-block norm columns & graph sel ---------------------------
    # normc[c]: [128, 4*NB]: kinds (0: -norm2, 1: -2*norm2, 2: inv_norm, 3: norm)
    normc = np.zeros((NCORES, P, 4 * NB), dtype=np.float32)
    gsel = np.full((NCORES, P, NB), -1.0, dtype=np.float32)
    for c in range(NCORES):
        lo = c * NLOC
        hi = min(lo + NLOC, N)
        n = hi - lo
        pad = NB * P - n
        nn = np.pad(norm[lo:hi], (0, pad)).reshape(NB, P).T
        n2 = np.pad(norm2[lo:hi], (0, pad)).reshape(NB, P).T
        iv = np.pad(inv_norm[lo:hi], (0, pad)).reshape(NB, P).T
        normc[c][:, 0 * NB : 1 * NB] = -n2
        normc[c][:, 1 * NB : 2 * NB] = -2.0 * n2
        normc[c][:, 2 * NB : 3 * NB] = iv
        normc[c][:, 3 * NB : 4 * NB] = nn
        gs = np.pad(graph_ids[lo:hi].astype(np.float32), (0, pad), constant_values=-1.0)
        gsel[c] = gs.reshape(NB, P).T

    block_rows = [min(P, NLOC - b * P) for b in range(NB)]

    return dict(
        N=N, E=E, G=G, NLOC=NLOC, NB=NB, NBATCH=NBATCH, NCH=NCH,
        TOT=TOT, NSUB=NSUB, calls=calls, pairs=pairs,
        first_pair=first_pair, last_pair=last_pair,
        sub_call=sub_call, sub_col=sub_col,
        idx16=idx16, dstsel=dstsel, normc=normc, gsel=gsel,
        block_rows=block_rows, norm=norm,
    )


DIN = [128, 128, 128, 256]
DOUT = [128, 128, 256, 512]


def _build(S):
    """Build the SPMD Bass program (shared across cores)."""
    KSTAGE = int(os.environ.get("KSTAGE", "99"))
    KSINGLE = os.environ.get("KSINGLE", "0") == "1"
    NLOC, NB, NCH, TOT = S["NLOC"], S["NB"], S["NCH"], S["TOT"]
    NPAIRS = len(S["pairs"])
    NTAB = NCORES * NLOC  # table rows (>= N)

    nc = bacc.Bacc(trn_type="TRN2", num_devices=1 if KSINGLE else NCORES,
                   dynamic_dma_scratch_size=49152, num_swdge_queues=4)

    sig_in = nc.dram_tensor("sig", [NLOC, 128], F32, kind="ExternalInput")
    idx_in = nc.dram_tensor("idx16", [P, TOT // 16], I16, kind="ExternalInput")
    dsel_in = nc.dram_tensor("dstsel", [P, NPAIRS], F32, kind="ExternalInput")
    normc_in = nc.dram_tensor("normc", [P, 4 * NB], F32, kind="ExternalInput")
    gsel_in = nc.dram_tensor("gsel", [P, NB], F32, kind="ExternalInput")
    iota_in = nc.dram_tensor("iota", [P, P], BF, kind="ExternalInput")
    ident_in = nc.dram_tensor("ident", [P, P], BF, kind="ExternalInput")
    w_in = [
        nc.dram_tensor(f"W{l}", [3 * DIN[l], DOUT[l]], BF, kind="ExternalInput")
        for l in range(4)
    ]
    bt_in = [
        nc.dram_tensor(f"Bt{l}", [P, DOUT[l]], F32, kind="ExternalInput")
        for l in range(4)
    ]
    wm1_in = nc.dram_tensor("Wm1", [512, 512], BF, kind="ExternalInput")
    bm1_in = nc.dram_tensor("Bm1", [P, 512], F32, kind="ExternalInput")
    wm2_in = nc.dram_tensor("Wm2", [512, 16], BF, kind="ExternalInput")
    bm2_in = nc.dram_tensor("Bm2", [P, 16], F32, kind="ExternalInput")
    out = nc.dram_tensor("out", [P, 16], F32, kind="ExternalOutput")

    with tile.TileContext(nc) as tc:
        with (
            tc.tile_pool(name="dram", bufs=1, space="DRAM") as dram,
            tc.tile_pool(name="res", bufs=1) as res,
            tc.tile_pool(name="sb", bufs=3) as sb,
            tc.tile_pool(name="scp", bufs=1, space="PSUM") as scp,
            tc.tile_pool(name="pp", bufs=2, space="PSUM") as pp,
            tc.tile_pool(name="tpp", bufs=1, space="PSUM") as tpp,
            tc.tile_pool(name="rdp", bufs=1, space="PSUM") as rdp,
        ):
            # ------- resident metadata -------
            idx_sb = res.tile([P, TOT // 16], I16)
            dsel_sb = res.tile([P, NPAIRS], F32)
            normc_sb = res.tile([P, 4 * NB], F32)
            gsel_sb = res.tile([P, NB], F32)
            iota_sb = res.tile([P, P], BF)
            ident_sb = res.tile([P, P], BF)
            nc.sync.dma_start(out=idx_sb[:], in_=idx_in[:, :])
            nc.sync.dma_start(out=dsel_sb[:], in_=dsel_in[:, :])
            nc.sync.dma_start(out=normc_sb[:], in_=normc_in[:, :])
            nc.sync.dma_start(out=gsel_sb[:], in_=gsel_in[:, :])
            nc.sync.dma_start(out=iota_sb[:], in_=iota_in[:, :])
            nc.sync.dma_start(out=ident_sb[:], in_=ident_in[:, :])
            w_sb = []
            for l in range(4):
                nchk = 3 * DIN[l] // P
                t = res.tile([P, nchk * DOUT[l]], BF, tag=f"W{l}")
                for j in range(nchk):
                    nc.sync.dma_start(
                        out=t[:, j * DOUT[l] : (j + 1) * DOUT[l]],
                        in_=w_in[l][j * P : (j + 1) * P, :],
                    )
                w_sb.append(t)
            bt_sb = []
            for l in range(4):
                t = res.tile([P, DOUT[l]], F32, tag=f"Bt{l}")
                nc.sync.dma_start(out=t[:], in_=bt_in[l][:, :])
                bt_sb.append(t)
            wm1_sb = res.tile([P, 4 * 512], BF)
            for j in range(4):
                nc.sync.dma_start(
                    out=wm1_sb[:, j * 512 : (j + 1) * 512],
                    in_=wm1_in[j * P : (j + 1) * P, :],
                )
            bm1_sb = res.tile([P, 512], F32)
            nc.sync.dma_start(out=bm1_sb[:], in_=bm1_in[:, :])
            wm2_sb = res.tile([P, 4 * 16], BF)
            for j in range(4):
                nc.sync.dma_start(
                    out=wm2_sb[:, j * 16 : (j + 1) * 16],
                    in_=wm2_in[j * P : (j + 1) * P, :],
                )
            bm2_sb = res.tile([P, 16], F32)
            nc.sync.dma_start(out=bm2_sb[:], in_=bm2_in[:, :])

            # ------- DRAM tables -------
            y0s = [dram.tile([NLOC, DIN[l]], BF, tag=f"y0s{l}", name=f"y0s{l}") for l in range(4)]
            y1s = [dram.tile([NLOC, DIN[l]], BF, tag=f"y1s{l}", name=f"y1s{l}") for l in range(4)]
            y0f = [dram.tile([NTAB, DIN[l]], BF, tag=f"y0f{l}", name=f"y0f{l}", addr_space="Shared") for l in range(4)]
            y1f = [dram.tile([NTAB, DIN[l]], BF, tag=f"y1f{l}", name=f"y1f{l}", addr_space="Shared") for l in range(4)]

            RG = [list(range(NCORES))]

            def ag(shard, full):
                if KSINGLE:
                    nc.sync.dma_start(out=full[0 : shard.shape[0], :], in_=shard[:])
                    return
                nc.gpsimd.collective_compute(
                    "AllGather", mybir.AluOpType.bypass,
                    replica_groups=RG, ins=[shard[:]], outs=[full[:]],
                )

            def ncol(kind, b):
                return normc_sb[:, kind * NB + b : kind * NB + b + 1]

            # ------- P0: Y0_0 = signal * norm -------
            for b in range(NB):
                rows = S["block_rows"][b]
                sg = sb.tile([P, 128], F32, tag="sg")
                if rows < P:
                    nc.any.memset(sg[:], 0.0)
                nc.sync.dma_start(out=sg[:rows], in_=sig_in[b * P : b * P + rows, :])
                o = sb.tile([P, 128], BF, tag="p0o")
                nc.any.tensor_scalar(
                    out=o[:], in0=sg[:], scalar1=ncol(3, b), scalar2=None,
                    op0=mybir.AluOpType.mult,
                )
                nc.sync.dma_start(out=y0s[0][b * P : b * P + rows, :], in_=o[:rows])
            ag(y0s[0], y0f[0])

            def dump_and_stop(tab):
                d = sb.tile([P, 16], BF, tag="dmp", name="dmp")
                nc.sync.dma_start(out=d[:], in_=tab[0:P, 0:16])
                d32 = sb.tile([P, 16], F32, tag="dmp32", name="dmp32")
                nc.any.tensor_copy(out=d32[:], in_=d[:])
                nc.sync.dma_start(out=out[:, :], in_=d32[:])

            stopped = KSTAGE == 0
            if stopped:
                dump_and_stop(y0f[0])

            # ------- scatter unit -------
            def scatter_unit(table, D, evac_fn):
                g_tiles = {}
                psums = {}
                emitted = -1
                for j, (sidx, b) in enumerate(S["pairs"]):
                    k = int(S["sub_call"][sidx])
                    if k > emitted:
                        for kk in range(emitted + 1, k + 1):
                            ch, o, L = S["calls"][kk]
                            rows_ch = min(CHUNK, NTAB - ch * CHUNK)
                            g = sb.tile([P, (MAX_CALL // P) * 256], BF, tag="g", bufs=4, name="g")
                            nc.gpsimd.dma_gather(
                                out_ap=g[:, : (L // P) * D].rearrange(
                                    "p (k d) -> p k d", d=D
                                ),
                                in_ap=table[ch * CHUNK : ch * CHUNK + rows_ch, :],
                                idxs_ap=idx_sb[:, o // 16 : (o + L) // 16],
                                num_idxs=L,
                                num_idxs_reg=L,
                                elem_size=D,
                                queue_num=kk % 4,
                            )
                            g_tiles[kk] = g
                        emitted = k
                    col = int(S["sub_col"][sidx])
                    s_t = sb.tile([P, P], BF, tag="s", bufs=8, name="s_t")
                    if j % 3 == 0:
                        # ACT path: onehot = Relu(1 - Abs(dsel - iota))
                        a_t = sb.tile([P, P], BF, tag="oha", bufs=4, name="a_t")
                        nc.scalar.activation(
                            a_t[:], iota_sb[:],
                            mybir.ActivationFunctionType.Abs,
                            bias=dsel_sb[:, j : j + 1], scale=-1.0,
                        )
                        nc.scalar.activation(
                            s_t[:], a_t[:],
                            mybir.ActivationFunctionType.Relu,
                            bias=1.0, scale=-1.0,
                        )
                    else:
                        nc.vector.tensor_scalar(
                            out=s_t[:], in0=iota_sb[:],
                            scalar1=dsel_sb[:, j : j + 1], scalar2=None,
                            op0=mybir.AluOpType.is_equal,
                        )
                    if b not in psums:
                        psums[b] = scp.tile([P, D], F32, tag=f"sc{b % BB}", name=f"scps{b % BB}")
                    nc.tensor.matmul(
                        out=psums[b][:],
                        lhsT=s_t[:],
                        rhs=g_tiles[k][:, col * D : (col + 1) * D],
                        start=(j == S["first_pair"][b]),
                        stop=(j == S["last_pair"][b]),
                    )
                    if j == S["last_pair"][b]:
                        evac_fn(b, psums.pop(b))

            # ------- layers -------
            psum_r = rdp.tile([P, 512], F32, tag="rd")
            for l in range(4):
                if stopped:
                    break
                D = DIN[l]
                nkc = D // P

                def evac_a(b, ps, l=l):
                    rows = S["block_rows"][b]
                    ev = sb.tile([P, D], BF, tag="ev")
                    nc.any.tensor_scalar(
                        out=ev[:], in0=ps[:], scalar1=ncol(0, b), scalar2=None,
                        op0=mybir.AluOpType.mult,
                    )
                    nc.sync.dma_start(
                        out=y1s[l][b * P : b * P + rows, :], in_=ev[:rows]
                    )

                scatter_unit(y0f[l], D, evac_a)
                ag(y1s[l], y1f[l])
                if KSTAGE == 10 + l:
                    dump_and_stop(y1f[l])
                    stopped = True
                    break

                def evac_b(b, ps, l=l, nkc=nkc, D=D):
                    rows = S["block_rows"][b]
                    tb = sb.tile([P, D], BF, tag="tb")
                    nc.any.tensor_scalar(
                        out=tb[:], in0=ps[:], scalar1=ncol(1, b), scalar2=None,
                        op0=mybir.AluOpType.mult,
                    )
                    y0b = sb.tile([P, D], BF, tag="y0b")
                    y1b = sb.tile([P, D], BF, tag="y1b")
                    if rows < P:
                        nc.any.memset(y0b[:], 0.0)
                        nc.any.memset(y1b[:], 0.0)
                    nc.sync.dma_start(
                        out=y0b[:rows], in_=y0s[l][b * P : b * P + rows, :]
                    )
                    nc.sync.dma_start(
                        out=y1b[:rows], in_=y1s[l][b * P : b * P + rows, :]
                    )
                    y2b = sb.tile([P, D], BF, tag="y2b")
                    nc.any.tensor_tensor(
                        out=y2b[:], in0=tb[:], in1=y0b[:],
                        op=mybir.AluOpType.subtract,
                    )
                    # transposes -> feature-major lhsT chunks
                    yts = []
                    for term, ysrc in enumerate((y0b, y1b, y2b)):
                        for kc in range(nkc):
                            tp = tpp.tile([P, P], BF, tag="tp")
                            nc.tensor.transpose(
                                out=tp[:],
                                in_=ysrc[:, kc * P : (kc + 1) * P],
                                identity=ident_sb[:],
                            )
                            yt = sb.tile([P, P], BF, tag="yt", bufs=8, name="yt")
                            nc.any.tensor_copy(out=yt[:], in_=tp[:])
                            yts.append(yt)
                    ph = pp.tile([P, DOUT[l]], F32, tag="dh")
                    nchk = 3 * nkc
                    for j2 in range(nchk):
                        nc.tensor.matmul(
                            out=ph[:],
                            lhsT=yts[j2][:],
                            rhs=w_sb[l][:, j2 * DOUT[l] : (j2 + 1) * DOUT[l]],
                            start=(j2 == 0),
                            stop=(j2 == nchk - 1),
                        )
                    t1 = sb.tile([P, DOUT[l]], F32, tag="t1")
                    nc.any.tensor_scalar(
                        out=t1[:], in0=ph[:], scalar1=ncol(2, b), scalar2=None,
                        op0=mybir.AluOpType.mult,
                    )
                    t2 = sb.tile([P, DOUT[l]], F32, tag="t2")
                    nc.any.tensor_tensor(
                        out=t2[:], in0=t1[:], in1=bt_sb[l][:],
                        op=mybir.AluOpType.add,
                    )
                    if l < 3:
                        o = sb.tile([P, DOUT[l]], BF, tag="lo")
                        nc.scalar.activation(
                            o[:], t2[:], mybir.ActivationFunctionType.Relu,
                            scale=ncol(3, b),
                        )
                        nc.sync.dma_start(
                            out=y0s[l + 1][b * P : b * P + rows, :], in_=o[:rows]
                        )
                    else:
                        h = sb.tile([P, 512], BF, tag="h")
                        nc.scalar.activation(
                            h[:], t2[:], mybir.ActivationFunctionType.Relu,
                        )
                        gs = sb.tile([P, P], BF, tag="gs")
                        nc.any.tensor_scalar(
                            out=gs[:], in0=iota_sb[:],
                            scalar1=gsel_sb[:, b : b + 1], scalar2=None,
                            op0=mybir.AluOpType.is_equal,
                        )
                        nc.tensor.matmul(
                            out=psum_r[:],
                            lhsT=gs[:],
                            rhs=h[:],
                            start=(b == 0),
                            stop=(b == NB - 1),
                        )

                scatter_unit(y1f[l], D, evac_b)
                if l < 3:
                    ag(y0s[l + 1], y0f[l + 1])
                if KSTAGE == 20 + l:
                    if l < 3:
                        dump_and_stop(y0f[l + 1])
                    stopped = True
                    break

            # ------- readout + MLP -------
            if stopped:
                hgp = None
            else:
                hgp = sb.tile([P, 512], F32, tag="hgp")
            if not stopped:
                nc.any.tensor_copy(out=hgp[:], in_=psum_r[:])
                part_d = dram.tile([P, 512], F32, tag="part")
                tot_d = dram.tile([P, 512], F32, tag="tot")
                nc.sync.dma_start(out=part_d[:], in_=hgp[:])
                if KSINGLE:
                    nc.sync.dma_start(out=tot_d[:], in_=part_d[:])
                else:
                    nc.gpsimd.collective_compute(
                        "AllReduce", mybir.AluOpType.add,
                        replica_groups=RG, ins=[part_d[:]], outs=[tot_d[:]],
                    )
                hg = sb.tile([P, 512], F32, tag="hg")
                nc.sync.dma_start(out=hg[:], in_=tot_d[:])
                hgb = sb.tile([P, 512], BF, tag="hgb")
                nc.any.tensor_copy(out=hgb[:], in_=hg[:])

            def mm_block(x_sb, wtile, dout, bias_tile, relu):
                # out = act(x @ W + b): x [128, 512] bf16 -> 4 transposed chunks
                xts = []
                for kc in range(4):
                    tp = tpp.tile([P, P], BF, tag="tp")
                    nc.tensor.transpose(
                        out=tp[:], in_=x_sb[:, kc * P : (kc + 1) * P],
                        identity=ident_sb[:],
                    )
                    xt = sb.tile([P, P], BF, tag="yt", bufs=8, name="xt")
                    nc.any.tensor_copy(out=xt[:], in_=tp[:])
                    xts.append(xt)
                ph = pp.tile([P, dout], F32, tag="dh")
                for kc in range(4):
                    nc.tensor.matmul(
                        out=ph[:], lhsT=xts[kc][:],
                        rhs=wtile[:, kc * dout : (kc + 1) * dout],
                        start=(kc == 0), stop=(kc == 3),
                    )
                odt = BF if relu else F32
                o = sb.tile([P, dout], odt, tag=f"mo{dout}")
                nc.any.tensor_tensor(
                    out=o[:], in0=ph[:], in1=bias_tile[:], op=mybir.AluOpType.add
                )
                if relu:
                    r = sb.tile([P, dout], BF, tag=f"mr{dout}")
                    nc.scalar.activation(
                        r[:], o[:], mybir.ActivationFunctionType.Relu
                    )
                    return r
                return o

            if not stopped:
                m1 = mm_block(hgb, wm1_sb, 512, bm1_sb, relu=True)
                m2 = mm_block(m1, wm2_sb, 16, bm2_sb, relu=False)
                nc.sync.dma_start(out=out[:, :], in_=m2[:])

    nc.finalize()
    return nc


_CACHE = {}
G_OVERRIDE = None      # test hook (reference uses G=128)
TRACE = False          # test hook: request NTFF profiling
LAST_RESULTS = None    # test hook: BassKernelResults of the last run


def kernel(signal, W0, b0, W1, b1, W2, b2, W3, b3, Wm1, bm1, Wm2, bm2,
           src, dst, graph_ids):
    global LAST_RESULTS
    signal = np.ascontiguousarray(np.asarray(signal, dtype=np.float32))
    src = np.asarray(src).astype(np.int64)
    dst = np.asarray(dst).astype(np.int64)
    graph_ids = np.asarray(graph_ids).astype(np.int64)
    N = signal.shape[0]
    G = G_OVERRIDE or 128

    key = (N, src.shape[0], G, hash(src.tobytes()) ^ hash(dst.tobytes())
           ^ hash(graph_ids.tobytes()))
    if key in _CACHE:
        S, nc = _CACHE[key]
    else:
        S = _preprocess(src, dst, graph_ids, N, G)
        nc = _build(S)
        _CACHE.clear()
        _CACHE[key] = (S, nc)

    in_maps = _make_inmaps(
        S, signal, W0, b0, W1, b1, W2, b2, W3, b3, Wm1, bm1, Wm2, bm2
    )

    res = run_bass_kernel_spmd(
        nc, in_maps, core_ids=list(range(NCORES)), trace=TRACE
    )
    LAST_RESULTS = res
    return np.asarray(res.results[0]["out"][:G, :10])


def _make_inmaps(S, signal, W0, b0, W1, b1, W2, b2, W3, b3, Wm1, bm1, Wm2, bm2):
    import ml_dtypes

    bf16 = ml_dtypes.bfloat16
    N = signal.shape[0]
    NLOC = S["NLOC"]
    iota_np = np.broadcast_to(
        np.arange(P, dtype=np.float32)[None, :], (P, P)
    ).astype(bf16)
    ident_np = np.eye(P, dtype=np.float32).astype(bf16)
    ws = [np.asarray(w, dtype=np.float32).astype(bf16) for w in (W0, W1, W2, W3)]
    bts = [
        np.broadcast_to(np.asarray(b, dtype=np.float32)[None, :], (P, len(b))).copy()
        for b in (b0, b1, b2, b3)
    ]
    wm2_p = np.zeros((512, 16), np.float32)
    wm2_p[:, :10] = np.asarray(Wm2, dtype=np.float32)
    bm2_p = np.zeros((P, 16), np.float32)
    bm2_p[:, :10] = np.asarray(bm2, dtype=np.float32)[None, :]
    bm1_t = np.broadcast_to(
        np.asarray(bm1, dtype=np.float32)[None, :], (P, 512)
    ).copy()

    in_maps = []
    for c in range(NCORES):
        lo = c * NLOC
        hi = min(N, lo + NLOC)
        shard = np.zeros((NLOC, 128), np.float32)
        shard[: hi - lo] = signal[lo:hi]
        m = {
            "sig": shard,
            "idx16": S["idx16"][c],
            "dstsel": S["dstsel"][c],
            "normc": S["normc"][c],
            "gsel": S["gsel"][c],
            "iota": iota_np,
            "ident": ident_np,
            "Wm1": np.asarray(Wm1, dtype=np.float32).astype(bf16),
            "Bm1": bm1_t,
            "Wm2": wm2_p.astype(bf16),
            "Bm2": bm2_p,
        }
        for l in range(4):
            m[f"W{l}"] = ws[l]
            m[f"Bt{l}"] = bts[l]
        in_maps.append(m)
    return in_maps

